# revision 41
# baseline (speedup 1.0000x reference)
"""Trainium2 Bass kernel for y = x @ W.T + b  (x: [16384,1024] f32,
W: [1024,1024] f32, b: [1024] f32) on 8 NeuronCores.

Data-parallel: x is split along batch into 8 shards of 2048 rows;
W and b are replicated. Each core computes its y shard with bf16
matmuls accumulating in fp32 PSUM; bias is fused into the PSUM->SBUF
eviction on the Scalar engine, which downcasts to bf16 (host upcasts
back to f32 -- halves store traffic; adds ~3e-3 rel err, well under
the gate). Host-side we pre-transpose x (and W) to put the contraction
dim on SBUF partitions, so no on-chip transposes are needed, and group
DRAM layouts so every DMA is 128 long contiguous runs.

Schedule per core (bq = one of 4 batch chunks of 512 rows):
- All input DMAs go on the Sync HWDGE ring in consumption order:
  (w[ko], x[bq0,ko]) pairs, bias, then x[bq1..3] in half-chunks. The
  first pair lands ~3.3 us after the body starts; pair cadence
  (~1.35 us) stays ahead of phase-A consumption (1.73 us/chunk warm).
- N=128 dummy matmuls emitted BEFORE the TileContext run from ~7.0 us
  (inside the framework preamble's shadow -- the tile entry handshake
  does not serialize DMA issue behind the tensor queue), so the HAM
  clock-gate window starts early and the PE un-throttles before the
  first real matmul. The bridge to data must be near GAP-FREE while
  still cold: a >~1.5 us PE idle gap resets the HAM ramp (measured:
  a 2.4 us gap delayed full clock by 5 us, and a mid-stream DMA stall
  re-throttled an already-warm PE). The PE warm clock varies with the
  chassis power state: measured 2.4 GHz (216 ns per N=512 bf16
  matmul) and 2.0 GHz (259 ns) in different sessions; cold is half.
  The matmul stream itself is at the issue floor in both states --
  per-MM overhead is ~0 (verified: weight reuse, LDWEIGHTS dedup, and
  per-MM semaphore stripping all left the cadence unchanged).
- bq0 runs contraction-outer across all 8 PSUM banks, consuming the
  pair stream as it arrives; bq1..3 run output-tile-outer (one PSUM
  bank at a time).
- Output tiles are evicted (bias fused, bf16 downcast) alternating
  between the Vector and Scalar engines, and stored on the ring
  opposite the evict engine, so no queue does evict+store
  back-to-back and the 8-bank burst at the end of bq0 drains in two
  parallel chains. The very last tile is split 256+128+128 so the
  final chain (evict -> 128-descriptor store issue -> HBM receipt)
  hangs off a narrow N=128 piece.
"""

import sys

if "/opt/trn_rl_repo" not in sys.path:
    sys.path.insert(0, "/opt/trn_rl_repo")

import ml_dtypes
import numpy as np

# concourse's trace path imports antenv.axon_hooks, which this image lacks.
# Register a working NTFF-profile hook (via the axon PJRT .so) so tracing
# works when requested, degrading to no-op if anything is missing.
try:
    import antenv.axon_hooks  # noqa: F401
except ImportError:
    import types as _types

    def _make_hook():
        try:
            from trn_agent_boot.trn_boot import _ntff_profile_via_ctypes

            return _ntff_profile_via_ctypes("/opt/axon/libaxon_pjrt.so")
        except Exception:
            return None

    _hooks = _types.ModuleType("antenv.axon_hooks")
    _hooks.get_axon_ntff_profile_hook = _make_hook
    _hooks.set_axon_ntff_profile_hook = lambda h: None
    sys.modules["antenv.axon_hooks"] = _hooks

BATCH = 16384
IN_F = 1024
OUT_F = 1024
NCORES = 8
P = 128
KO = IN_F // P  # 8 contraction tiles
MO = OUT_F // P  # 8 output-feature tiles
BS = BATCH // NCORES  # 2048 rows per core
FD = 512  # matmul moving free dim (one PSUM bank of fp32)
NB = BS // FD  # 4 batch chunks per core
# N=128 dummy matmuls bridging from body start (~7.5 us) until the
# first (w0, x00) pair lands (~10.7-11.5 us). Sized to just reach the
# data with NO multi-us PE idle gap: a big gap resets the HAM
# clock-gate ramp and costs ~3-5 us of half-clock matmuls. Sub-us
# undershoot is tolerated (measured: a 0.85 us gap did not reset it).
N_WARM = 0
# Raw (pre-TileContext) dummy matmuls: the tensor queue starts
# executing at ~3-5 us (after the runtime preamble + library load),
# well before the tile entry handshake at ~7.4 us. Matmuls emitted
# before the TileContext run in that window, so the PE's HAM activity
# window starts counting ~2 us earlier and the clock un-throttles
# before the first real matmul. They read garbage SBUF (values are
# irrelevant and discarded; the body reuses the PSUM bank with
# start=True). Sized to end near the handshake so they neither leave
# a >1.5 us PE idle gap nor delay the handshake much.
N_PRE = 30

_cache = {}
LAST_RESULT = None


def _build():
    import concourse.mybir as mybir
    import concourse.tile as tile
    from concourse import bacc, bass

    nc = bacc.Bacc(None, target_bir_lowering=False)
    # xT4[p, bq, ko, fd] = x[bq*FD + fd, ko*P + p]
    xT = nc.declare_dram_parameter(
        "xT", [P, NB, KO, FD], mybir.dt.bfloat16, isOutput=False
    )
    # w3[p, ko, mo, c] = W[mo*P + c, ko*P + p]  (ko-major: bq0 consumes
    # weights one ko chunk at a time). NOTE: w and x must stay SEPARATE
    # SBUF tiles -- packing them into one tile puts the PE's two SBUF
    # read ports (weights + moving operand) on the same sub-banks and
    # slows every matmul ~20%.
    w3 = nc.declare_dram_parameter(
        "w3", [P, KO, MO, P], mybir.dt.bfloat16, isOutput=False
    )
    bias = nc.declare_dram_parameter("bias", [P, MO], mybir.dt.float32, isOutput=False)
    # out4[p, bq, mo, fd] = y[bq*FD + fd, mo*P + p]
    out = nc.declare_dram_parameter(
        "out", [P, NB, MO, FD], mybir.dt.bfloat16, isOutput=True
    )

    # Pre-TileContext HAM warm-up (see N_PRE comment). The SBUF tensor
    # stays allocated through the tile context (ExitStack closed after),
    # so nothing aliases it; the PSUM bank is freed before the tile pool
    # claims all 8 banks, and the body's start=True matmuls clear it.
    import contextlib

    _pre = contextlib.ExitStack()
    wu_pre = _pre.enter_context(nc.sbuf_tensor("wu_pre", [P, P], mybir.dt.bfloat16))
    ap2 = [[P, P], [1, P]]
    # NOTE: a pre-tile "pilot" DMA to absorb SDMA ring spin-up latency
    # was measured useless-to-harmful: the w0 transfer landed ~1 us
    # earlier, but the completion SEMAPHORE (what the matmuls wait on)
    # fired no sooner, and the shifted issue slots reintroduced ~3 us
    # of phase-A chunk stalls. The data-ready path is bound by DMA
    # completion-notification latency, not ring spin-up.
    with nc.psum_tensor("wu_pre_ps", [P, P], mybir.dt.float32) as wu_pre_ps:
        for _ in range(N_PRE):
            nc.tensor.matmul(
                bass.AP(wu_pre_ps, 0, ap2),
                bass.AP(wu_pre, 0, ap2),
                bass.AP(wu_pre, 0, ap2),
                start=True,
                stop=True,
            )

    with tile.TileContext(nc) as tc:
        with (
            tc.tile_pool(name="const", bufs=1) as cpool,
            tc.tile_pool(name="outp", bufs=3) as opool,
            tc.tile_pool(name="psum", bufs=8, space="PSUM") as ppool,
        ):
            x_sb = cpool.tile([P, NB, KO, FD], mybir.dt.bfloat16)
            w_sb = cpool.tile([P, KO, MO, P], mybir.dt.bfloat16)
            b_sb = cpool.tile([P, MO], mybir.dt.float32)
            if N_WARM:
                wu_sb = cpool.tile([P, P], mybir.dt.bfloat16)
                # memset on GpSimd: it has no other body work, so the
                # first warm-up isn't queued behind Vector's preamble.
                nc.gpsimd.memset(wu_sb[:], 0.0)
            # All inputs on ONE HWDGE ring (sync), in consumption order.
            # A single ring streams (w, x) ko-pairs at ~1.35 us cadence;
            # phase A consumes a chunk in 1.73 us (2.4 GHz warm) or more,
            # so the stream stays ahead with no stalls. Splitting w onto a
            # second ring was measured WORSE: cross-ring SDMA interleaving
            # slowed the w stream enough to stall phase A ~4 us waiting on
            # late w chunks, and the first pair landed no earlier. FIFO
            # order also guarantees arrival order == consumption order.
            # NOTE: splitting w0 into halves (to start the first matmuls
            # ~0.3 us earlier) was measured WORSE: the extra issue slot
            # delays later pair chunks, phase A's thin DMA margin
            # (~0.35 us/chunk at 2.4 GHz) flips into a multi-us stall,
            # and that stall re-throttles the PE clock (HAM MID window).
            # w chunks after w0 are packed two-per-DMA: each DMA_DIRECT2D
            # costs ~0.6-0.8 us of ring issue time (128 per-partition
            # descriptors) almost independent of size, and per-chunk
            # (w, x) issue time (~1.46 us) exceeded the transfer cadence
            # (~1.35 us), making issue the binding constraint. Packing
            # drops issue to ~1.17 us/chunk, restoring transfer-limited
            # streaming with more margin for phase A.
            # w0..w3 stay unpacked: with the PE warm from the first
            # real matmul (pre-tile warm-up), phase A consumes a chunk
            # All w chunks unpacked: packing two ko-chunks per DMA (to
            # save ring issue slots) was measured to always stall phase A
            # at the FIRST packed transfer (1.9 us for w12, 0.7 us for
            # w23, 1.2 us for w45 as packing was pushed later) -- the
            # 512 KiB transfer lump lands right at the warm consumption
            # deadline. Issue pressure never materialized as time.
            for k in range(KO):
                nc.sync.dma_start(w_sb[:, k], w3[:, k])
                nc.sync.dma_start(x_sb[:, 0, k], xT[:, 0, k])
            # bias on the (idle) scalar ring: frees a sync issue slot so
            # the x1 half-chunks issue ~0.6 us earlier -- boundary margin
            # for slow-DMA runs. Bias isn't needed until the first
            # eviction (~24 us), so its ring and timing are free.
            nc.scalar.dma_start(b_sb[:], bias[:])
            # Bulk x in half-chunks: phase B consumes ko-ascending, so
            # landing x1[ko0-3] ~3 us earlier removes a ~1 us stall at
            # the phase A -> B boundary (measured: phase B's first MM
            # waited on the full-1-MiB x1 transfer).
            for bq in range(1, NB):
                h = KO // 2
                nc.sync.dma_start(x_sb[:, bq, 0:h], xT[:, bq, 0:h])
                nc.sync.dma_start(x_sb[:, bq, h:KO], xT[:, bq, h:KO])

            # Optional in-body warm-up stub (N_WARM=0: the pre-tile
            # warm-ups already bridge past the entry handshake and HAM
            # is warm before data lands; body dummies would only delay
            # the first real matmul by ~0.1 us each).
            if N_WARM:
                wu_ps = ppool.tile([P, FD], mybir.dt.float32, tag="ps")
                for _ in range(N_WARM):
                    nc.tensor.matmul(
                        wu_ps[:, :P], wu_sb[:], wu_sb[:], start=True, stop=True
                    )

            # bq0: contraction-outer over all 8 PSUM banks, consuming
            # (w, x) ko-chunks in DMA arrival order.
            ps0 = [
                ppool.tile([P, FD], mybir.dt.float32, tag="ps", name=f"ps0_{mo}")
                for mo in range(MO)
            ]
            o_sb = opool.tile([P, MO, FD], mybir.dt.bfloat16)
            for ko in range(KO):
                for mo in range(MO):
                    nc.tensor.matmul(
                        ps0[mo][:],
                        w_sb[:, ko, mo],
                        x_sb[:, 0, ko],
                        start=(ko == 0),
                        stop=(ko == KO - 1),
                    )
            # Evictions alternate Vector/Scalar so the 8-bank burst at the
            # end of bq0 drains in two parallel ~2.4 us chains instead of
            # one ~5.5 us chain (whose backlog stalled later bank reuse).
            for mo in range(MO):
                if mo % 2 == 0:
                    nc.vector.tensor_scalar_add(
                        o_sb[:, mo], ps0[mo][:], b_sb[:, mo : mo + 1]
                    )
                else:
                    nc.scalar.activation(
                        o_sb[:, mo],
                        ps0[mo][:],
                        mybir.ActivationFunctionType.Identity,
                        bias=b_sb[:, mo : mo + 1],
                    )
                # Stores alternate rings, opposite to the evict engine,
                # so neither queue does evict+store back-to-back.
                st = nc.scalar if mo % 2 == 0 else nc.sync
                st.dma_start(out[:, 0, mo], o_sb[:, mo])

            # bq1..3: output-tile-outer, one PSUM bank at a time;
            # evict + store each tile as soon as it completes.
            for bq in range(1, NB):
                o_sb = opool.tile([P, MO, FD], mybir.dt.bfloat16)
                for mo in range(MO):
                    if bq == NB - 1 and mo == MO - 1:
                        # Very last tile: split 256+128+128 so the final
                        # dependency chain (evict -> 128-descriptor store
                        # issue -> HBM receipt) hangs off a narrow N=128
                        # piece. Evicts and stores alternate engines/rings
                        # so the three chains overlap.
                        pieces = [(0, 256), (256, 128), (384, 128)]
                        ev_eng = ["scalar", "vector", "vector"]
                        # Last piece stores via the scalar ring: piece 1's
                        # issue clears it before piece 2's evict lands,
                        # while the sync ring is still busy with piece 0's
                        # store (+ a branch) at that moment (measured
                        # ~0.5 us queue delay on the final store).
                        st_eng = [nc.sync, nc.scalar, nc.scalar]
                        for h, (off, width) in enumerate(pieces):
                            hs = slice(off, off + width)
                            ps = ppool.tile(
                                [P, FD], mybir.dt.float32, tag="ps", name=f"ps_l{h}"
                            )
                            for ko in range(KO):
                                nc.tensor.matmul(
                                    ps[:, :width],
                                    w_sb[:, ko, mo],
                                    x_sb[:, bq, ko, hs],
                                    start=(ko == 0),
                                    stop=(ko == KO - 1),
                                )
                            if ev_eng[h] == "scalar":
                                nc.scalar.activation(
                                    o_sb[:, mo, hs],
                                    ps[:, :width],
                                    mybir.ActivationFunctionType.Identity,
                                    bias=b_sb[:, mo : mo + 1],
                                )
                            else:
                                nc.vector.tensor_scalar_add(
                                    o_sb[:, mo, hs], ps[:, :width], b_sb[:, mo : mo + 1]
                                )
                            st_eng[h].dma_start(out[:, bq, mo, hs], o_sb[:, mo, hs])
                    else:
                        ps = ppool.tile([P, FD], mybir.dt.float32, tag="ps")
                        for ko in range(KO):
                            nc.tensor.matmul(
                                ps[:],
                                w_sb[:, ko, mo],
                                x_sb[:, bq, ko],
                                start=(ko == 0),
                                stop=(ko == KO - 1),
                            )
                        if mo % 2 == 0:
                            nc.vector.tensor_scalar_add(
                                o_sb[:, mo], ps[:], b_sb[:, mo : mo + 1]
                            )
                        else:
                            nc.scalar.activation(
                                o_sb[:, mo],
                                ps[:],
                                mybir.ActivationFunctionType.Identity,
                                bias=b_sb[:, mo : mo + 1],
                            )
                        st = nc.scalar if mo % 2 == 0 else nc.sync
                        st.dma_start(out[:, bq, mo], o_sb[:, mo])

    _pre.close()
    nc.compile()
    return nc


def kernel(x, weight, bias):
    global LAST_RESULT
    from concourse.bass_utils import run_bass_kernel_spmd

    if "nc" not in _cache:
        _cache["nc"] = _build()
    nc = _cache["nc"]

    x = np.asarray(x, dtype=np.float32)
    weight = np.asarray(weight, dtype=np.float32)
    bias = np.asarray(bias, dtype=np.float32)

    bf16 = ml_dtypes.bfloat16
    # w3[p, ko, mo, c] = W[mo*P + c, ko*P + p]
    wb = weight.astype(bf16).reshape(MO, P, KO, P)  # [mo, c, ko, p]
    w3 = np.ascontiguousarray(wb.transpose(3, 2, 0, 1))  # [p, ko, mo, c]
    # bias laid out [P, MO]: b[p, mo] = bias[mo*P + p]
    b_t = np.ascontiguousarray(bias.astype(np.float32).reshape(MO, P).T)

    in_maps = []
    for c in range(NCORES):
        xs = x[c * BS : (c + 1) * BS].astype(bf16)
        # xT4[p, bq, ko, fd] = x[bq*FD + fd, ko*P + p]
        xr = xs.reshape(NB, FD, KO, P)  # [bq, fd, ko, p]
        xT = np.ascontiguousarray(xr.transpose(3, 0, 2, 1))  # [p, bq, ko, fd]
        in_maps.append({"xT": xT, "w3": w3, "bias": b_t})

    res = run_bass_kernel_spmd(nc, in_maps, list(range(NCORES)))
    LAST_RESULT = res

    y = np.empty((BATCH, OUT_F), dtype=np.float32)
    for c in range(NCORES):
        o = res.results[c]["out"]  # [p, bq, mo, fd] bf16
        y[c * BS : (c + 1) * BS] = (
            o.astype(np.float32).transpose(1, 3, 2, 0).reshape(BS, OUT_F)
        )
    return y



# revision 42
# speedup vs baseline: 1.0029x; 1.0029x over previous
"""Trainium2 Bass kernel for y = x @ W.T + b  (x: [16384,1024] f32,
W: [1024,1024] f32, b: [1024] f32) on 8 NeuronCores.

Data-parallel: x is split along batch into 8 shards of 2048 rows;
W and b are replicated. Each core computes its y shard with bf16
matmuls accumulating in fp32 PSUM; bias is fused into the PSUM->SBUF
eviction on the Scalar engine, which downcasts to bf16 (host upcasts
back to f32 -- halves store traffic; adds ~3e-3 rel err, well under
the gate). Host-side we pre-transpose x (and W) to put the contraction
dim on SBUF partitions, so no on-chip transposes are needed, and group
DRAM layouts so every DMA is 128 long contiguous runs.

Schedule per core (bq = one of 4 batch chunks of 512 rows):
- All input DMAs go on the Sync HWDGE ring in consumption order:
  (w[ko], x[bq0,ko]) pairs, bias, then x[bq1..3] in half-chunks. The
  first pair lands ~3.3 us after the body starts; pair cadence
  (~1.35 us) stays ahead of phase-A consumption (1.73 us/chunk warm).
- N=128 dummy matmuls emitted BEFORE the TileContext run from ~7.0 us
  (inside the framework preamble's shadow -- the tile entry handshake
  does not serialize DMA issue behind the tensor queue), so the HAM
  clock-gate window starts early and the PE un-throttles before the
  first real matmul. The bridge to data must be near GAP-FREE while
  still cold: a >~1.5 us PE idle gap resets the HAM ramp (measured:
  a 2.4 us gap delayed full clock by 5 us, and a mid-stream DMA stall
  re-throttled an already-warm PE). The PE warm clock varies with the
  chassis power state: measured 2.4 GHz (216 ns per N=512 bf16
  matmul) and 2.0 GHz (259 ns) in different sessions; cold is half.
  The matmul stream itself is at the issue floor in both states --
  per-MM overhead is ~0 (verified: weight reuse, LDWEIGHTS dedup, and
  per-MM semaphore stripping all left the cadence unchanged).
- bq0 runs contraction-outer across all 8 PSUM banks, consuming the
  pair stream as it arrives; bq1..3 run output-tile-outer (one PSUM
  bank at a time).
- Output tiles are evicted (bias fused, bf16 downcast) alternating
  between the Vector and Scalar engines, and stored on the ring
  opposite the evict engine, so no queue does evict+store
  back-to-back and the 8-bank burst at the end of bq0 drains in two
  parallel chains. The very last tile is split 256+128+128 so the
  final chain (evict -> 128-descriptor store issue -> HBM receipt)
  hangs off a narrow N=128 piece.
"""

import sys

if "/opt/trn_rl_repo" not in sys.path:
    sys.path.insert(0, "/opt/trn_rl_repo")

import ml_dtypes
import numpy as np

# concourse's trace path imports antenv.axon_hooks, which this image lacks.
# Register a working NTFF-profile hook (via the axon PJRT .so) so tracing
# works when requested, degrading to no-op if anything is missing.
try:
    import antenv.axon_hooks  # noqa: F401
except ImportError:
    import types as _types

    def _make_hook():
        try:
            from trn_agent_boot.trn_boot import _ntff_profile_via_ctypes

            return _ntff_profile_via_ctypes("/opt/axon/libaxon_pjrt.so")
        except Exception:
            return None

    _hooks = _types.ModuleType("antenv.axon_hooks")
    _hooks.get_axon_ntff_profile_hook = _make_hook
    _hooks.set_axon_ntff_profile_hook = lambda h: None
    sys.modules["antenv.axon_hooks"] = _hooks

BATCH = 16384
IN_F = 1024
OUT_F = 1024
NCORES = 8
P = 128
KO = IN_F // P  # 8 contraction tiles
MO = OUT_F // P  # 8 output-feature tiles
BS = BATCH // NCORES  # 2048 rows per core
FD = 512  # matmul moving free dim (one PSUM bank of fp32)
NB = BS // FD  # 4 batch chunks per core
# N=128 dummy matmuls bridging from body start (~7.5 us) until the
# first (w0, x00) pair lands (~10.7-11.5 us). Sized to just reach the
# data with NO multi-us PE idle gap: a big gap resets the HAM
# clock-gate ramp and costs ~3-5 us of half-clock matmuls. Sub-us
# undershoot is tolerated (measured: a 0.85 us gap did not reset it).
N_WARM = 0
# Raw (pre-TileContext) dummy matmuls: the tensor queue starts
# executing at ~3-5 us (after the runtime preamble + library load),
# well before the tile entry handshake at ~7.4 us. Matmuls emitted
# before the TileContext run in that window, so the PE's HAM activity
# window starts counting ~2 us earlier and the clock un-throttles
# before the first real matmul. They read garbage SBUF (values are
# irrelevant and discarded; the body reuses the PSUM bank with
# start=True). Sized to end near the handshake so they neither leave
# a >1.5 us PE idle gap nor delay the handshake much.
N_PRE = 30

_cache = {}
LAST_RESULT = None


def _build():
    import concourse.mybir as mybir
    import concourse.tile as tile
    from concourse import bacc, bass

    nc = bacc.Bacc(None, target_bir_lowering=False)
    # xT4[p, bq, ko, fd] = x[bq*FD + fd, ko*P + p]
    xT = nc.declare_dram_parameter(
        "xT", [P, NB, KO, FD], mybir.dt.bfloat16, isOutput=False
    )
    # w3[p, ko, mo, c] = W[mo*P + c, ko*P + p]  (ko-major: bq0 consumes
    # weights one ko chunk at a time). NOTE: w and x must stay SEPARATE
    # SBUF tiles -- packing them into one tile puts the PE's two SBUF
    # read ports (weights + moving operand) on the same sub-banks and
    # slows every matmul ~20%.
    w3 = nc.declare_dram_parameter(
        "w3", [P, KO, MO, P], mybir.dt.bfloat16, isOutput=False
    )
    bias = nc.declare_dram_parameter("bias", [P, MO], mybir.dt.float32, isOutput=False)
    # out4[p, bq, mo, fd] = y[bq*FD + fd, mo*P + p]
    out = nc.declare_dram_parameter(
        "out", [P, NB, MO, FD], mybir.dt.bfloat16, isOutput=True
    )

    # Pre-TileContext HAM warm-up (see N_PRE comment). The SBUF tensor
    # stays allocated through the tile context (ExitStack closed after),
    # so nothing aliases it; the PSUM bank is freed before the tile pool
    # claims all 8 banks, and the body's start=True matmuls clear it.
    import contextlib

    _pre = contextlib.ExitStack()
    wu_pre = _pre.enter_context(nc.sbuf_tensor("wu_pre", [P, P], mybir.dt.bfloat16))
    ap2 = [[P, P], [1, P]]
    # NOTE: a pre-tile "pilot" DMA to absorb SDMA ring spin-up latency
    # was measured useless-to-harmful: the w0 transfer landed ~1 us
    # earlier, but the completion SEMAPHORE (what the matmuls wait on)
    # fired no sooner, and the shifted issue slots reintroduced ~3 us
    # of phase-A chunk stalls. The data-ready path is bound by DMA
    # completion-notification latency, not ring spin-up.
    with nc.psum_tensor("wu_pre_ps", [P, P], mybir.dt.float32) as wu_pre_ps:
        for _ in range(N_PRE):
            nc.tensor.matmul(
                bass.AP(wu_pre_ps, 0, ap2),
                bass.AP(wu_pre, 0, ap2),
                bass.AP(wu_pre, 0, ap2),
                start=True,
                stop=True,
            )

    with tile.TileContext(nc) as tc:
        with (
            tc.tile_pool(name="const", bufs=1) as cpool,
            tc.tile_pool(name="outp", bufs=3) as opool,
            tc.tile_pool(name="psum", bufs=8, space="PSUM") as ppool,
        ):
            x_sb = cpool.tile([P, NB, KO, FD], mybir.dt.bfloat16)
            w_sb = cpool.tile([P, KO, MO, P], mybir.dt.bfloat16)
            b_sb = cpool.tile([P, MO], mybir.dt.float32)
            if N_WARM:
                wu_sb = cpool.tile([P, P], mybir.dt.bfloat16)
                # memset on GpSimd: it has no other body work, so the
                # first warm-up isn't queued behind Vector's preamble.
                nc.gpsimd.memset(wu_sb[:], 0.0)
            # All inputs on ONE HWDGE ring (sync), in consumption order.
            # A single ring streams (w, x) ko-pairs at ~1.35 us cadence;
            # phase A consumes a chunk in 1.73 us (2.4 GHz warm) or more,
            # so the stream stays ahead with no stalls. Splitting w onto a
            # second ring was measured WORSE: cross-ring SDMA interleaving
            # slowed the w stream enough to stall phase A ~4 us waiting on
            # late w chunks, and the first pair landed no earlier. FIFO
            # order also guarantees arrival order == consumption order.
            # NOTE: splitting w0 into halves (to start the first matmuls
            # ~0.3 us earlier) was measured WORSE: the extra issue slot
            # delays later pair chunks, phase A's thin DMA margin
            # (~0.35 us/chunk at 2.4 GHz) flips into a multi-us stall,
            # and that stall re-throttles the PE clock (HAM MID window).
            # w chunks after w0 are packed two-per-DMA: each DMA_DIRECT2D
            # costs ~0.6-0.8 us of ring issue time (128 per-partition
            # descriptors) almost independent of size, and per-chunk
            # (w, x) issue time (~1.46 us) exceeded the transfer cadence
            # (~1.35 us), making issue the binding constraint. Packing
            # drops issue to ~1.17 us/chunk, restoring transfer-limited
            # streaming with more margin for phase A.
            # w0..w3 stay unpacked: with the PE warm from the first
            # real matmul (pre-tile warm-up), phase A consumes a chunk
            # All w chunks unpacked: packing two ko-chunks per DMA (to
            # save ring issue slots) was measured to always stall phase A
            # at the FIRST packed transfer (1.9 us for w12, 0.7 us for
            # w23, 1.2 us for w45 as packing was pushed later) -- the
            # 512 KiB transfer lump lands right at the warm consumption
            # deadline. Issue pressure never materialized as time.
            for k in range(KO):
                nc.sync.dma_start(w_sb[:, k], w3[:, k])
                nc.sync.dma_start(x_sb[:, 0, k], xT[:, 0, k])
            nc.sync.dma_start(b_sb[:], bias[:])
            # Bulk x in half-chunks: phase B consumes ko-ascending, so
            # landing x1[ko0-3] ~3 us earlier removes a ~1 us stall at
            # the phase A -> B boundary (measured: phase B's first MM
            # waited on the full-1-MiB x1 transfer).
            for bq in range(1, NB):
                h = KO // 2
                nc.sync.dma_start(x_sb[:, bq, 0:h], xT[:, bq, 0:h])
                nc.sync.dma_start(x_sb[:, bq, h:KO], xT[:, bq, h:KO])

            # Optional in-body warm-up stub (N_WARM=0: the pre-tile
            # warm-ups already bridge past the entry handshake and HAM
            # is warm before data lands; body dummies would only delay
            # the first real matmul by ~0.1 us each).
            if N_WARM:
                wu_ps = ppool.tile([P, FD], mybir.dt.float32, tag="ps")
                for _ in range(N_WARM):
                    nc.tensor.matmul(
                        wu_ps[:, :P], wu_sb[:], wu_sb[:], start=True, stop=True
                    )

            # bq0: contraction-outer over all 8 PSUM banks, consuming
            # (w, x) ko-chunks in DMA arrival order.
            ps0 = [
                ppool.tile([P, FD], mybir.dt.float32, tag="ps", name=f"ps0_{mo}")
                for mo in range(MO)
            ]
            o_sb = opool.tile([P, MO, FD], mybir.dt.bfloat16)
            for ko in range(KO):
                for mo in range(MO):
                    nc.tensor.matmul(
                        ps0[mo][:],
                        w_sb[:, ko, mo],
                        x_sb[:, 0, ko],
                        start=(ko == 0),
                        stop=(ko == KO - 1),
                    )
            # Evictions alternate Vector/Scalar so the 8-bank burst at the
            # end of bq0 drains in two parallel ~2.4 us chains instead of
            # one ~5.5 us chain (whose backlog stalled later bank reuse).
            for mo in range(MO):
                if mo % 2 == 0:
                    nc.vector.tensor_scalar_add(
                        o_sb[:, mo], ps0[mo][:], b_sb[:, mo : mo + 1]
                    )
                else:
                    nc.scalar.activation(
                        o_sb[:, mo],
                        ps0[mo][:],
                        mybir.ActivationFunctionType.Identity,
                        bias=b_sb[:, mo : mo + 1],
                    )
                # Stores alternate rings, opposite to the evict engine,
                # so neither queue does evict+store back-to-back.
                st = nc.scalar if mo % 2 == 0 else nc.sync
                st.dma_start(out[:, 0, mo], o_sb[:, mo])

            # bq1..3: output-tile-outer, one PSUM bank at a time;
            # evict + store each tile as soon as it completes.
            for bq in range(1, NB):
                o_sb = opool.tile([P, MO, FD], mybir.dt.bfloat16)
                for mo in range(MO):
                    if bq == NB - 1 and mo == MO - 1:
                        # Very last tile: split 256+128+128 so the final
                        # dependency chain (evict -> 128-descriptor store
                        # issue -> HBM receipt) hangs off a narrow N=128
                        # piece. Evicts and stores alternate engines/rings
                        # so the three chains overlap.
                        pieces = [(0, 256), (256, 128), (384, 128)]
                        ev_eng = ["scalar", "vector", "vector"]
                        # Last piece stores via the scalar ring: piece 1's
                        # issue clears it before piece 2's evict lands,
                        # while the sync ring is still busy with piece 0's
                        # store (+ a branch) at that moment (measured
                        # ~0.5 us queue delay on the final store).
                        st_eng = [nc.sync, nc.scalar, nc.scalar]
                        for h, (off, width) in enumerate(pieces):
                            hs = slice(off, off + width)
                            ps = ppool.tile(
                                [P, FD], mybir.dt.float32, tag="ps", name=f"ps_l{h}"
                            )
                            for ko in range(KO):
                                nc.tensor.matmul(
                                    ps[:, :width],
                                    w_sb[:, ko, mo],
                                    x_sb[:, bq, ko, hs],
                                    start=(ko == 0),
                                    stop=(ko == KO - 1),
                                )
                            if ev_eng[h] == "scalar":
                                nc.scalar.activation(
                                    o_sb[:, mo, hs],
                                    ps[:, :width],
                                    mybir.ActivationFunctionType.Identity,
                                    bias=b_sb[:, mo : mo + 1],
                                )
                            else:
                                nc.vector.tensor_scalar_add(
                                    o_sb[:, mo, hs], ps[:, :width], b_sb[:, mo : mo + 1]
                                )
                            st_eng[h].dma_start(out[:, bq, mo, hs], o_sb[:, mo, hs])
                    else:
                        ps = ppool.tile([P, FD], mybir.dt.float32, tag="ps")
                        for ko in range(KO):
                            nc.tensor.matmul(
                                ps[:],
                                w_sb[:, ko, mo],
                                x_sb[:, bq, ko],
                                start=(ko == 0),
                                stop=(ko == KO - 1),
                            )
                        if mo % 2 == 0:
                            nc.vector.tensor_scalar_add(
                                o_sb[:, mo], ps[:], b_sb[:, mo : mo + 1]
                            )
                        else:
                            nc.scalar.activation(
                                o_sb[:, mo],
                                ps[:],
                                mybir.ActivationFunctionType.Identity,
                                bias=b_sb[:, mo : mo + 1],
                            )
                        st = nc.scalar if mo % 2 == 0 else nc.sync
                        st.dma_start(out[:, bq, mo], o_sb[:, mo])

    _pre.close()
    nc.compile()
    return nc


def kernel(x, weight, bias):
    global LAST_RESULT
    from concourse.bass_utils import run_bass_kernel_spmd

    if "nc" not in _cache:
        _cache["nc"] = _build()
    nc = _cache["nc"]

    x = np.asarray(x, dtype=np.float32)
    weight = np.asarray(weight, dtype=np.float32)
    bias = np.asarray(bias, dtype=np.float32)

    bf16 = ml_dtypes.bfloat16
    # w3[p, ko, mo, c] = W[mo*P + c, ko*P + p]
    wb = weight.astype(bf16).reshape(MO, P, KO, P)  # [mo, c, ko, p]
    w3 = np.ascontiguousarray(wb.transpose(3, 2, 0, 1))  # [p, ko, mo, c]
    # bias laid out [P, MO]: b[p, mo] = bias[mo*P + p]
    b_t = np.ascontiguousarray(bias.astype(np.float32).reshape(MO, P).T)

    in_maps = []
    for c in range(NCORES):
        xs = x[c * BS : (c + 1) * BS].astype(bf16)
        # xT4[p, bq, ko, fd] = x[bq*FD + fd, ko*P + p]
        xr = xs.reshape(NB, FD, KO, P)  # [bq, fd, ko, p]
        xT = np.ascontiguousarray(xr.transpose(3, 0, 2, 1))  # [p, bq, ko, fd]
        in_maps.append({"xT": xT, "w3": w3, "bias": b_t})

    res = run_bass_kernel_spmd(nc, in_maps, list(range(NCORES)))
    LAST_RESULT = res

    y = np.empty((BATCH, OUT_F), dtype=np.float32)
    for c in range(NCORES):
        o = res.results[c]["out"]  # [p, bq, mo, fd] bf16
        y[c * BS : (c + 1) * BS] = (
            o.astype(np.float32).transpose(1, 3, 2, 0).reshape(BS, OUT_F)
        )
    return y

